# revision 47
# baseline (speedup 1.0000x reference)
"""Bass/Trainium2 kernel for DenseAtt: out = sigmoid(x@w_i [:,None] + x@w_j [None,:] + b).

Sharding: rows of the (8192, 8192) output are split across 8 NeuronCores
(1024 rows each). Every core receives the full x (bf16, host-transposed to
[feat, rows]) plus its local row block, computes its row block, and the host
concatenates + upcasts. 118.3us (f32 stores, single DMA queue) -> 52.9us.

Design, driven by the CoreSim v1 cost model that grades this kernel:
  * A DMA instruction costs free_bytes_per_partition * 0.3855ns ON ITS
    ISSUING ENGINE's queue (x2 if the contiguous element run < 512B,
    min 500ns), so DMA bandwidth scales with the number of issuing queues.
    SP, Activation and Pool (gpsimd/SWDGE) can all issue DMAs: the 50.5us
    of bf16 output stores are split across SP and Pool, with loads and the
    tiny rearrange DMAs placed in each queue's ramp slack.
  * Output is stored as bf16 (~0.2% rel err vs the 2e-2 budget): halves
    store traffic vs f32.
  * x is shipped bf16 AND pre-transposed on the host, so xT [feat, rows]
    chunks load at 4KB/partition descriptors with no on-chip transpose.
  * The 8M-element pointwise sigmoid would be one-engine-bound, so it is
    split ACT:20 / DVE:12 row-tiles:
      - A-tiles: sigmoid on ACT straight out of PSUM (pb = b_full
        replicated across partitions by a wj-broadcast matmul; bias = the
        per-partition a column). pb is single-buffered (PSUM is exactly
        full); its refill is emitted at a schedule point where the in-order
        PE queue reaches it just as the last sigmoid drains it.
      - D-tiles: sigma(z) = 1/(1 + e^-a e^-b): PE computes w = 1 + u_i v_j
        into PSUM with a K=2 matmul ([u;1]^T [v;1]) and DVE does a single
        IEEE reciprocal pass.
  * u = e^-(a+c), v = e^-b are derived in tiny [128,16] column space from
    s = sigmoid(-z) as s*(1/(1-s)) (sigmoid-table only: Sigmoid and Exp
    never share an ACT table set, and DVE has no divide), then two
    strided-column PE transposes -> [n/2, 256] staging (512B runs dodge
    the sub-512B 2x DMA descriptor penalty) -> one rearrange DMA into the
    K=2 operand rows. All four v rows live in distinct free-dim quarters
    of one [2, 8192] tile (they are all produced during segment 0 and must
    not alias) and are computed up front, so segments 1-3 run with zero
    v-chain coupling; segment 0 is A-tile-heavy since its D path waits
    ~10us for the u/v chains.
  * The Sigmoid ACT table is pre-loaded by a dummy activation at t=0, and
    the final tile is stored in halves on both DMA queues to shorten the
    kernel tail.
"""

import numpy as np

_N = 8192          # rows/cols of the output
_D = 128           # feature dim
_M = 8             # cores
_R = _N // _M      # 1024 rows per core
_SEG = 2048        # output column segment width
_NSEG = _N // _SEG # 4 segments
_NT = _R // 128    # 8 row tiles per core
_CT = _SEG // 128  # 16 column tiles per segment (v-chain granularity)

# per-segment row-tile schedule: (rt, path, store queue); vk = position
# after which the next segment's v-chain is emitted (None = skip).
# path A = ACT sigmoid from pb, D = PE K=2 matmul + DVE reciprocal.
# Segments end on a D tile so pb frees early for the next segment's
# matmuls; seg 3 is D-first / A-last so ACT and DVE drain together.
_SCHED = [
    # seg 0: mostly A-tiles (the D path waits on the u/v prologue chains,
    # ~10us); the v-chains for segments 1-3 are emitted after A0/A1/A2,
    # landing in ACT/PE/DVE slack while sigma tiles run
    [(0, "A", "sp"), (1, "A", "pool"), (2, "A", "sp"), (3, "A", "pool"),
     (6, "D", "pool"), (4, "A", "sp"), (7, "D", "pool"), (5, "A", "pool")],
    [(0, "A", "sp"), (5, "D", "pool"), (1, "A", "pool"), (6, "D", "sp"),
     (2, "A", "pool"), (7, "D", "pool"), (3, "A", "sp"), (4, "A", "pool")],
    [(0, "A", "sp"), (5, "D", "pool"), (1, "A", "pool"), (6, "D", "sp"),
     (2, "A", "pool"), (7, "D", "pool"), (3, "A", "sp"), (4, "A", "pool")],
    # seg 3: D-leaning first, A-last so ACT and DVE drain together
    [(5, "D", "pool"), (0, "A", "sp"), (6, "D", "pool"), (1, "A", "sp"),
     (7, "D", "pool"), (2, "A", "pool"), (4, "D", "sp"), (3, "A", "sp")],
]

_nc_cache = None


def _split_multi_waits(nc, mybir, max_keep=1):
    """Walrus on this toolchain only encodes ONE sem wait per instruction
    (NEURON_ISA_TPB_EVENTS has a single wait slot); Tile emits multi-wait
    sync_info. Split extras onto NoOps inserted right before the instruction
    on the same engine."""
    n_split = 0
    for fn in nc.m.functions:
        for bb in fn.blocks:
            newlist = []
            changed = False
            for inst in list(bb.instructions):
                si = inst.sync_info
                if si is not None and si.on_wait and len(si.on_wait) > max_keep:
                    waits = list(si.on_wait)
                    extra, keep = waits[:-max_keep], waits[-max_keep:]
                    for k, w in enumerate(extra):
                        newlist.append(
                            mybir.InstNoOp(
                                name=f"{inst.name}-waitsplit{k}",
                                engine=inst.engine,
                                sync_info=mybir.SyncInfo(on_wait=[w], on_update=[]),
                                bass_nofuse=True,
                            )
                        )
                        n_split += 1
                    inst.sync_info = mybir.SyncInfo(
                        on_wait=keep, on_update=list(si.on_update)
                    )
                    changed = True
                newlist.append(inst)
            if changed:
                bb.instructions = newlist
    return n_split


def _build():
    global _nc_cache
    if _nc_cache is not None:
        return _nc_cache

    import concourse.bass as bass
    import concourse.mybir as mybir
    from concourse.tile import TileContext

    f32 = mybir.dt.float32
    bf16 = mybir.dt.bfloat16
    Sigmoid = mybir.ActivationFunctionType.Sigmoid
    Op = mybir.AluOpType

    nc = bass.Bass("TRN2", debug=False, num_devices=_M)

    # x transposed on host: [feat, rows]
    xtb_d = nc.dram_tensor("xtb", [_D, _N], bf16, kind="ExternalInput")
    # local row block transposed on host: [feat, local rows]
    xltb_d = nc.dram_tensor("xltb", [_D, _R], bf16, kind="ExternalInput")
    # bf16 constants: [:, :128] = wj_rep (w_j down each column), [:, 128] = w_i,
    # [:, 129] = w_j
    cstb_d = nc.dram_tensor("cstb", [_D, _D + 2], bf16, kind="ExternalInput")
    # f32 constants: [:, 0] = linear bias b replicated, [:, 1:129] = eye(128)
    cstf_d = nc.dram_tensor("cstf", [_D, _D + 1], f32, kind="ExternalInput")
    out_d = nc.dram_tensor("out", [_R, _N], bf16, kind="ExternalOutput")

    with TileContext(nc) as tc, nc.allow_low_precision(
        reason="bf16 tiles are the final store precision"
    ):
        with (
            tc.tile_pool(name="const", bufs=1) as cpool,
            tc.tile_pool(name="sm", bufs=4) as smpool,
            tc.tile_pool(name="st", bufs=3) as stpool,
            tc.tile_pool(name="outp", bufs=12) as opool,
            tc.tile_pool(name="pb", bufs=1, space="PSUM") as pb_pool,
            tc.tile_pool(name="pw", bufs=2, space="PSUM") as w_pool,
        ):
            q = {"sp": nc.sync, "act": nc.scalar, "pool": nc.gpsimd}

            # ACT queue: cstf then a dummy sigmoid to pre-load the ACT table
            # off the critical path
            cstf = cpool.tile([128, _D + 1], f32)
            nc.scalar.dma_start(out=cstf[:], in_=cstf_d[:])
            warm = cpool.tile([128, 1], f32)
            nc.vector.memset(warm[:], 0.0)
            warm_o = cpool.tile([128, 1], f32)
            nc.scalar.activation(warm_o[:], warm[:], Sigmoid)

            # SP queue: bf16 constants, then chunk 0 (split for an earlier
            # first matmul), then the local block
            cstb = cpool.tile([128, _D + 2], bf16)
            nc.gpsimd.dma_start(out=cstb[:], in_=cstb_d[:])
            xT = cpool.tile([128, _N], bf16)    # x transposed [feat, rows]
            nc.sync.dma_start(out=xT[:, 0:1024], in_=xtb_d[:, 0:1024])
            nc.sync.dma_start(out=xT[:, 1024:_SEG], in_=xtb_d[:, 1024:_SEG])
            # Pool carries the other half of the ramp loads in parallel
            xlT = cpool.tile([128, _R], bf16)   # local block transposed
            nc.gpsimd.dma_start(out=xlT[:], in_=xltb_d[:])

            wj_rep = cstb[:, 0:_D]
            wi = cstb[:, _D:_D + 1]
            wj = cstb[:, _D + 1:_D + 2]
            bias_col = cstf[:, 0:1]
            eye = cstf[:, 1:_D + 1]

            # K=2 outer-product operands: uv4 row 0 holds each segment's v
            # row in its own free-dim quarter (all four are produced during
            # segment 0, so they must not alias), row 1 is all-ones.
            # u2 = (u, ones).
            uv4 = cpool.tile([2, _N], bf16)
            u2 = cpool.tile([2, _R], bf16)
            ones_st = cpool.tile([16, 256], bf16)
            nc.vector.memset(ones_st[:], 1.0)
            nc.sync.dma_start(out=uv4[1:2, 0:_N // 2], in_=ones_st[0:16, :])
            nc.gpsimd.dma_start(out=uv4[1:2, _N // 2:_N], in_=ones_st[0:16, :])
            nc.sync.dma_start(out=u2[1:2, :], in_=ones_st[0:4, :])

            def exp_neg_col(zcol, dst_row, n, dq):
                """Given z in column layout zcol [128, n] (n even; PSUM or
                SBUF), produce e^-z as a bf16 row [1, n*128] at dst_row:
                s = sigmoid(-z), e^-z = s/(1-s), two strided-column PE
                transposes into an [n/2, 256] f32 staging tile (512B runs,
                so the rearrange DMA on queue dq avoids the sub-512B 2x
                descriptor penalty), then one converting Pool/SP DMA."""
                s = smpool.tile([128, _CT], f32, tag="sm")
                nc.scalar.activation(s[:, 0:n], zcol, Sigmoid, scale=-1.0)
                t1 = smpool.tile([128, _CT], f32, tag="sm")
                nc.vector.tensor_scalar(
                    out=t1[:, 0:n], in0=s[:, 0:n], scalar1=-1.0, scalar2=1.0,
                    op0=Op.mult, op1=Op.add,
                )
                r1 = smpool.tile([128, _CT], f32, tag="sm")
                nc.vector.reciprocal(r1[:, 0:n], t1[:, 0:n])
                col = smpool.tile([128, _CT], f32, tag="sm")
                nc.vector.tensor_tensor(
                    out=col[:, 0:n], in0=s[:, 0:n], in1=r1[:, 0:n], op=Op.mult
                )
                h = n // 2
                pt = w_pool.tile([128, 1024], f32, tag="pw")
                nc.tensor.transpose(pt[0:h, 0:128], col[:, 0:n:2], eye)
                nc.tensor.transpose(pt[0:h, 128:256], col[:, 1:n:2], eye)
                st = stpool.tile([128, 256], bf16, tag="st")
                nc.vector.tensor_copy(out=st[0:h, :], in_=pt[0:h, 0:256])
                q[dq].dma_start(out=dst_row, in_=st[0:h, :])
                return st

            def v_chain(s, dq="pool"):
                """Column-space b -> e^-b row for segment s (b from xT),
                into pair s%2 of uv2. Emitted during segment s-1."""
                c0 = s * _SEG
                pa = w_pool.tile([128, 1024], f32, tag="pw")
                for t in range(_CT):
                    nc.tensor.matmul(
                        pa[:, t:t + 1],
                        xT[:, c0 + t * 128:c0 + (t + 1) * 128], wj,
                    )
                exp_neg_col(pa[:, 0:_CT], uv4[0:1, c0:c0 + _SEG], _CT, dq)

            # ---- a column: a = xl @ w_i + b (per-partition, [128, 8]) ----
            pa = w_pool.tile([128, 1024], f32, tag="pw")
            for t in range(_NT):
                nc.tensor.matmul(
                    pa[:, t:t + 1], xlT[:, t * 128:(t + 1) * 128], wi
                )
            a_col = cpool.tile([128, _NT], f32)
            nc.vector.tensor_scalar_add(
                out=a_col[:], in0=pa[:, 0:_NT], scalar1=bias_col
            )
            # u = e^-(a+b), duplicated at base partition 32 so lhsT matches
            # either uv2 ping-pong pair
            exp_neg_col(a_col[:], u2[0:1, :], _NT, "sp")
            # v row for segment 0
            v_chain(0, "sp")
            # remaining x chunks: needed by the v-chains emitted in seg 0
            for cs in range(1, _NSEG):
                nc.sync.dma_start(
                    out=xT[:, cs * _SEG:(cs + 1) * _SEG],
                    in_=xtb_d[:, cs * _SEG:(cs + 1) * _SEG],
                )

            # ---- seg-major main loop ----
            def fill_pb(s):
                pb = pb_pool.tile([128, _SEG], f32, tag="pb")
                c0 = s * _SEG
                for h in range(_SEG // 512):
                    nc.tensor.matmul(
                        pb[:, h * 512:(h + 1) * 512],
                        wj_rep, xT[:, c0 + h * 512:c0 + (h + 1) * 512],
                    )
                return pb

            pb = fill_pb(0)
            for s in range(_NSEG):
                c0 = s * _SEG
                last_a = max(k for k, t in enumerate(_SCHED[s]) if t[1] == "A")
                for k, (rt, path, sq) in enumerate(_SCHED[s]):
                    o = opool.tile([128, _SEG], bf16, tag="o")
                    if path == "A" and s == _NSEG - 1 and k == len(_SCHED[s]) - 1:
                        # final tile: sigmoid + store in halves on both DMA
                        # queues so the kernel tail is one half-store shorter
                        for h2 in range(2):
                            nc.scalar.activation(
                                o[:, h2 * 1024:(h2 + 1) * 1024],
                                pb[:, h2 * 1024:(h2 + 1) * 1024], Sigmoid,
                                bias=a_col[:, rt:rt + 1], scale=1.0,
                            )
                            q["sp" if h2 == 0 else "pool"].dma_start(
                                out=out_d[rt * 128:(rt + 1) * 128,
                                          c0 + h2 * 1024:c0 + (h2 + 1) * 1024],
                                in_=o[:, h2 * 1024:(h2 + 1) * 1024],
                            )
                        continue
                    if path == "A":
                        nc.scalar.activation(
                            o[:], pb[:], Sigmoid, bias=a_col[:, rt:rt + 1],
                            scale=1.0,
                        )
                    else:
                        for h in range(_SEG // 1024):
                            w = w_pool.tile([128, 1024], f32, tag="pw")
                            for g in range(2):
                                cw = h * 1024 + g * 512
                                nc.tensor.matmul(
                                    w[:, g * 512:(g + 1) * 512],
                                    u2[0:2, rt * 128:(rt + 1) * 128],
                                    uv4[0:2, c0 + cw:c0 + cw + 512],
                                )
                            nc.vector.reciprocal(
                                o[:, h * 1024:(h + 1) * 1024], w[:]
                            )
                    q[sq].dma_start(
                        out=out_d[rt * 128:(rt + 1) * 128, c0:c0 + _SEG],
                        in_=o[:],
                    )
                    if s == 0 and k < 3:
                        # v rows for segments 1-3, one per A-tile slot; all
                        # of them complete during segment 0, so segments 1-3
                        # run with zero v-chain coupling
                        v_chain(k + 1, "pool")
                    if k == last_a and s + 1 < _NSEG:
                        # refill pb for the next segment as soon as this
                        # segment's sigmoids have drained it
                        pb = fill_pb(s + 1)

    _split_multi_waits(nc, mybir)

    _nc_cache = nc
    return nc


_runner_cache = None


def _get_runner(nc):
    """Build (once) a jitted shard_map callable around the bass_exec custom
    call, so repeated kernel() calls skip the per-call retrace/recompile that
    run_bass_kernel_spmd's fresh closures would incur."""
    global _runner_cache
    if _runner_cache is not None:
        return _runner_cache

    import jax
    from jax.experimental.shard_map import shard_map
    from jax.sharding import Mesh, PartitionSpec
    from concourse import bass2jax
    import concourse.mybir as mybir

    bass2jax.install_neuronx_cc_hook()

    in_names, out_names, out_avals, zero_outs = [], [], [], []
    for alloc in nc.m.functions[0].allocations:
        if not isinstance(alloc, mybir.MemoryLocationSet):
            continue
        name = alloc.memorylocations[0].name
        if alloc.kind == "ExternalInput":
            in_names.append(name)
        elif alloc.kind == "ExternalOutput":
            out_names.append(name)
            shape = tuple(alloc.tensor_shape)
            dtype = mybir.dt.np(alloc.dtype)
            out_avals.append(jax.core.ShapedArray(shape, dtype))
            zero_outs.append(np.zeros(shape, dtype))

    partition_name = nc.partition_id_tensor.name if nc.partition_id_tensor else None
    if partition_name is not None:
        in_names = [n for n in in_names if n != partition_name]
    n_params = len(in_names)
    all_names = in_names + out_names
    if partition_name is not None:
        all_names = all_names + [partition_name]

    def _body(*args):
        operands = list(args)
        if partition_name is not None:
            operands.append(bass2jax.partition_id_tensor())
        outs = bass2jax._bass_exec_p.bind(
            *operands,
            out_avals=tuple(out_avals),
            in_names=tuple(all_names),
            out_names=tuple(out_names),
            lowering_input_output_aliases=(),
            sim_require_finite=True,
            sim_require_nnan=True,
            nc=nc,
        )
        return tuple(outs)

    devices = jax.devices()[:_M]
    mesh = Mesh(np.asarray(devices), ("core",))
    nspecs = n_params + len(out_names)
    fn = jax.jit(
        shard_map(
            _body,
            mesh=mesh,
            in_specs=(PartitionSpec("core"),) * nspecs,
            out_specs=(PartitionSpec("core"),) * len(out_names),
            check_rep=False,
        ),
        keep_unused=True,
    )
    # Stage the (all-zero) output operands on device once; without donation
    # they are never consumed, so every call reuses them instead of shipping
    # the zeros through the relay each time.
    from jax.sharding import NamedSharding

    sh = NamedSharding(mesh, PartitionSpec("core"))
    zeros_dev = [
        jax.device_put(np.zeros((_M * z.shape[0], *z.shape[1:]), z.dtype), sh)
        for z in zero_outs
    ]
    _runner_cache = (fn, in_names, zeros_dev)
    return _runner_cache


class _Res:
    exec_time_ns = None
    results = None
    mean_exec_time_ns = None
    max_exec_time_core_id = None
    instructions_and_trace = None


def _make_in_maps(inputs):
    import concourse.mybir as mybir

    bf16 = mybir.dt.np(mybir.dt.bfloat16)
    x = np.asarray(inputs["x"], dtype=np.float32)
    w = np.asarray(inputs["w"], dtype=np.float32)
    b = np.asarray(inputs["b"], dtype=np.float32)
    assert x.shape == (_N, _D), x.shape

    xt = np.ascontiguousarray(x.T.astype(bf16))          # [feat, rows]

    cstb = np.zeros((_D, _D + 2), dtype=np.float32)
    cstb[:, :_D] = w[0, _D:][:, None]        # wj_rep: w_j down each column
    cstb[:, _D] = w[0, :_D]                  # w_i
    cstb[:, _D + 1] = w[0, _D:]              # w_j
    cstb = np.ascontiguousarray(cstb.astype(bf16))

    cstf = np.zeros((_D, _D + 1), dtype=np.float32)
    cstf[:, 0] = b[0]
    cstf[:, 1:] = np.eye(_D, dtype=np.float32)

    return [
        {
            "xtb": xt,
            "xltb": np.ascontiguousarray(xt[:, c * _R:(c + 1) * _R]),
            "cstb": cstb,
            "cstf": cstf,
        }
        for c in range(_M)
    ]


def _run(inputs, trace=False, trace_cores=None):
    from concourse._compat import axon_active

    nc = _build()
    in_maps = _make_in_maps(inputs)

    if axon_active() and not trace:
        fn, in_names, zeros_dev = _get_runner(nc)
        args = [
            np.concatenate([m[name] for m in in_maps], axis=0) for name in in_names
        ] + list(zeros_dev)
        out_cat = np.asarray(fn(*args)[0])
        out = out_cat.reshape(_M * _R, _N).astype(np.float32)
        return _Res(), out

    from concourse.bass_utils import run_bass_kernel_spmd

    res = run_bass_kernel_spmd(
        nc, in_maps, core_ids=list(range(_M)), trace=trace, trace_cores=trace_cores
    )
    out = np.concatenate(
        [np.asarray(r["out"]).astype(np.float32) for r in res.results], axis=0
    )
    return res, out


def kernel(**inputs):
    _, out = _run(inputs)
    return out
